# revision 9
# baseline (speedup 1.0000x reference)
"""Trainium2 Bass kernel for nn_EquiCtsConvBase (equivariant continuous conv).

Math reformulation (per batch b, center m, field point n):
  rel = (field[n] - center[m]) / RADIUS
  r, theta = polar(rel)
  Bilinear grid-sample of kernel[(co,ci,y,x), theta_pad, r] at
  (gx, gy) decomposes into separable hat functions:
    Wx[j]  = relu(1 - |4r - 0.5 - j|)            j = 0..3   (radius cells)
    Wy[l]  = relu(1 - |iy - l|), iy = 4*theta/pi + 4.5, l = 0..9
  Circular theta padding folds 10 rows -> 8 bins:
    Wy8[0] = max(Wy[1], Wy[9]); Wy8[7] = max(Wy[0], Wy[8]); Wy8[b]=Wy[b+1]
  att = relu(1 - |rel|^2)^3 * mask[n]   (mask folded into feat on host)
  A[(b8,j), n, m] = relu(Wx_pre[j]) * relu(Wy8_pre[b8]) * att
  G[m, cell, f]   = sum_n A[cell, n, m] * feat[n, f]        (PE matmul 1)
  out[m, (co,y)]  = sum_{cell,f} G * K2[cell, f, (co,y)]    (PE matmul 2)
  out /= max(psi, tiny), psi[m] = sum_n att[n, m]  (extra ones-column matmul)

theta is computed without a Sqrt (keeps a single ACT table, trig_and_small):
  phi = arctan(rely/relx);  theta = phi + pi*sign(rely)*[relx<0]
  r   = |relx*sin(phi+pi/2) + rely*sin(phi)|

Sharding: 8 cores; core c handles batch b = c//4, centers m0 = (c%4)*96 .. +96.
Each core's SPMD program is identical; only input data differs.
"""

import math
import numpy as np

RADIUS = 1.5
B, M, N = 2, 384, 384
CI = CO = 8
M_LOC = 96          # centers per core
NCH = 3             # n-chunks of 128 (N = 384)
NCELL = 32          # 8 theta bins x 4 radius cells
FREE = NCH * M_LOC  # 288: fused (chunk, m) free dim for elementwise ops
N_CORES = 8

# --- engine assignment tuning knobs ---
CFG = dict(
    wy_act=tuple(range(0, 8)),   # Wy hat indices computed on ACT (rest on DVE)
    n_a_gps=10,                  # how many of the 32 A-cell ops go to GPSIMD
    wya_gps=True,                # wya (8 stt ops) on GPSIMD
    use_bcast=True,              # stride-0 free-dim broadcast reads
    a_dtype="f32",               # dtype for A / feat matmul operands
)

_module_cache = {}


def _build_module(cfg):
    import concourse.bass as bass
    import concourse.mybir as mybir
    from concourse import tile

    dt = mybir.dt
    Alu = mybir.AluOpType
    Act = mybir.ActivationFunctionType

    nc = bass.Bass("TRN2", target_bir_lowering=False, debug=False,
                   num_devices=N_CORES)

    # ------------- DRAM I/O -------------
    if cfg["use_bcast"]:
        cbd = nc.dram_tensor("cb", [128, 2 * M_LOC], dt.float32,
                             kind="ExternalInput").ap()      # cx|cy per m
        f3d = nc.dram_tensor("f3", [128, 2 * NCH], dt.float32,
                             kind="ExternalInput").ap()      # fx|fy per chunk
    else:
        cbd = nc.dram_tensor("cb", [128, 2 * FREE], dt.float32,
                             kind="ExternalInput").ap()
        f3d = nc.dram_tensor("f3", [128, 2 * FREE], dt.float32,
                             kind="ExternalInput").ap()
    featd = nc.dram_tensor("featx", [128, NCH * 17], dt.float32,
                           kind="ExternalInput").ap()
    k2d = nc.dram_tensor("k2b", [128, 4 * 16], dt.float32,
                         kind="ExternalInput").ap()
    cstd = nc.dram_tensor("cst", [128, 16], dt.float32,
                          kind="ExternalInput").ap()
    outd = nc.dram_tensor("out", [16, M_LOC], dt.float32,
                          kind="ExternalOutput").ap()

    f32 = dt.float32
    f32r = dt.float32r

    with tile.TileContext(nc) as tc:
        with tc.tile_pool(name="p", bufs=1) as pool, \
             tc.tile_pool(name="ps", bufs=1, space="PSUM") as psum:

            # ---------- loads ----------
            cb_s = pool.tile(list(cbd.shape), f32, tag="cb", name="cb_s")
            f3_s = pool.tile(list(f3d.shape), f32, tag="f3", name="f3_s")
            feat_s = pool.tile([128, NCH * 17], f32, tag="feat", name="feat_s")
            k2_s = pool.tile([128, 64], f32, tag="k2", name="k2_s")
            cst_s = pool.tile([128, 16], f32, tag="cst", name="cst_s")
            nc.sync.dma_start(cb_s[:], cbd[:])
            nc.sync.dma_start(f3_s[:], f3d[:])
            nc.sync.dma_start(feat_s[:], featd[:])
            nc.sync.dma_start(k2_s[:], k2d[:])
            nc.sync.dma_start(cst_s[:], cstd[:])

            # const bias columns: 0..9 -> -l (Wy), 10..13 -> -(0.5+j) (Wx),
            # 14 -> pi/2, 15 -> 1.0
            def cB(i):
                return cst_s[:, i:i + 1]

            def wt(tag, shape=None):
                return pool.tile(shape or [128, NCH, M_LOC], f32, tag=tag,
                                 name=tag)

            # broadcast views [128, NCH, M_LOC]
            if cfg["use_bcast"]:
                cx_b = cb_s[:, None, 0:M_LOC].to_broadcast((128, NCH, M_LOC))
                cy_b = cb_s[:, None, M_LOC:2 * M_LOC].to_broadcast(
                    (128, NCH, M_LOC))
                fx_b = f3_s[:, 0:NCH, None].to_broadcast((128, NCH, M_LOC))
                fy_b = f3_s[:, NCH:2 * NCH, None].to_broadcast(
                    (128, NCH, M_LOC))
            else:
                cx_b = cb_s[:, 0:FREE].rearrange("p (u m) -> p u m", u=NCH)
                cy_b = cb_s[:, FREE:2 * FREE].rearrange(
                    "p (u m) -> p u m", u=NCH)
                fx_b = f3_s[:, 0:FREE].rearrange("p (u m) -> p u m", u=NCH)
                fy_b = f3_s[:, FREE:2 * FREE].rearrange(
                    "p (u m) -> p u m", u=NCH)

            V, S, G = nc.vector, nc.scalar, nc.gpsimd

            # ---------- elementwise stage ----------
            relx = wt("relx"); rely = wt("rely")
            V.tensor_tensor(relx[:], fx_b, cx_b, Alu.subtract)
            V.tensor_tensor(rely[:], fy_b, cy_b, Alu.subtract)

            sqx = wt("sqx"); sqy = wt("sqy"); rho = wt("rho")
            S.activation(sqx[:], relx[:], Act.Square)
            S.activation(sqy[:], rely[:], Act.Square)
            V.tensor_tensor(rho[:], sqx[:], sqy[:], Alu.add)

            # phi = atan(rely/relx) with range reduction to |arg| <= 1:
            #   t1 = y/x, t2 = x/y; swap where |t1| > 1;
            #   phi = swap ? sign(y)*sign(x)*pi/2 - atan(t2) : atan(t1)
            rx = wt("rx"); ry = wt("ry"); t1 = wt("t1"); t2 = wt("t2")
            swp = wt("swp"); phi = wt("phi"); psw = wt("psw")
            sgn = wt("sgn"); neg = wt("neg"); sgx = wt("sgx"); sgt = wt("sgt")
            V.reciprocal(rx[:], relx[:])
            V.reciprocal(ry[:], rely[:])
            V.tensor_tensor(t1[:], rely[:], rx[:], Alu.mult)
            V.tensor_tensor(t2[:], relx[:], ry[:], Alu.mult)
            V.tensor_scalar(swp[:], t1[:], 0.0, 1.0, Alu.abs_max, Alu.is_gt)
            V.tensor_copy(phi[:], t1[:])
            V.copy_predicated(phi[:], swp[:], t2[:])
            S.activation(phi[:], phi[:], Act.Arctan)
            S.activation(sgn[:], rely[:], Act.Sign)
            V.tensor_scalar(neg[:], relx[:], 0.0, None, Alu.is_lt)
            V.tensor_scalar(sgx[:], neg[:], -2.0, 1.0, Alu.mult, Alu.add)
            V.tensor_tensor(sgt[:], sgn[:], sgx[:], Alu.mult)
            V.scalar_tensor_tensor(psw[:], sgt[:], math.pi / 2, phi[:],
                                   Alu.mult, Alu.subtract)
            V.copy_predicated(phi[:], swp[:], psw[:])

            cs = wt("cs"); sn = wt("sn")
            S.activation(cs[:], phi[:], Act.Sin, bias=cB(14))
            S.activation(sn[:], phi[:], Act.Sin)

            xc = wt("xc"); ys = wt("ys"); rr = wt("rr")
            V.tensor_tensor(xc[:], relx[:], cs[:], Alu.mult)
            V.tensor_tensor(ys[:], rely[:], sn[:], Alu.mult)
            V.tensor_tensor(rr[:], xc[:], ys[:], Alu.add)
            V.tensor_scalar(rr[:], rr[:], 0.0, None, Alu.abs_max)  # r = |.|

            corr = wt("corr"); phis = wt("phis"); iy = wt("iy")
            V.tensor_tensor(corr[:], sgn[:], neg[:], Alu.mult)
            V.tensor_scalar(phis[:], phi[:], 4.0 / math.pi, 4.5,
                            Alu.mult, Alu.add)
            V.scalar_tensor_tensor(iy[:], corr[:], 4.0, phis[:],
                                   Alu.mult, Alu.add)

            u1 = wt("u1"); u2 = wt("u2")
            S.activation(u1[:], rho[:], Act.Relu, bias=cB(15), scale=-1.0)
            V.tensor_tensor(u2[:], u1[:], u1[:], Alu.mult)

            # A tile: [128, 33 cells, NCH, M_LOC]; block 32 = att
            a_t = pool.tile([128, NCELL + 1, NCH, M_LOC], f32, tag="a_t", name="a_t")
            att = a_t[:, NCELL, :, :]
            V.scalar_tensor_tensor(att, u2[:], 1.0, u1[:],
                                   Alu.mult, Alu.mult)  # u1^3

            # Wy hats (pre-relu): wyh[l] = 1 - |iy - l|
            wyh = pool.tile([128, 10, NCH, M_LOC], f32, tag="wyh", name="wyh")
            for l in range(10):
                dst = wyh[:, l, :, :]
                if l in cfg["wy_act"]:
                    S.activation(dst, iy[:], Act.Abs, bias=cB(l))
                    S.activation(dst, dst, Act.Identity, bias=cB(15), scale=-1.0)
                else:
                    V.tensor_scalar(dst, iy[:], float(l), 0.0,
                                    Alu.subtract, Alu.abs_max)
                    V.tensor_scalar(dst, dst, -1.0, 1.0, Alu.mult, Alu.add)

            # fold to 8 bins (pre-relu; supports disjoint => max == sum)
            w0s = wt("w0s"); w7s = wt("w7s")
            V.tensor_tensor(w0s[:], wyh[:, 1, :, :], wyh[:, 9, :, :], Alu.max)
            V.tensor_tensor(w7s[:], wyh[:, 0, :, :], wyh[:, 8, :, :], Alu.max)

            def wy8_pre(b8):
                if b8 == 0:
                    return w0s[:]
                if b8 == 7:
                    return w7s[:]
                return wyh[:, b8 + 1, :, :]

            # wya[b8] = relu(wy8_pre) * att
            wya = pool.tile([128, 8, NCH, M_LOC], f32, tag="wya", name="wya")
            eng_wya = G if cfg["wya_gps"] else V
            for b8 in range(8):
                eng_wya.scalar_tensor_tensor(wya[:, b8, :, :], wy8_pre(b8),
                                             0.0, att, Alu.max, Alu.mult)

            # Wx pre-relu hats on ACT: wxp[j] = 1 - |4r - (0.5+j)|
            wxp = pool.tile([128, 4, NCH, M_LOC], f32, tag="wxp", name="wxp")
            for j in range(4):
                dst = wxp[:, j, :, :]
                S.activation(dst, rr[:], Act.Abs, bias=cB(10 + j), scale=4.0)
                S.activation(dst, dst, Act.Identity, bias=cB(15), scale=-1.0)

            # A cells: A = relu(wx_pre) * wya
            n_gps = cfg["n_a_gps"]
            for cell in range(NCELL):
                b8, j = divmod(cell, 4)
                eng = G if cell < n_gps else V
                eng.scalar_tensor_tensor(a_t[:, cell, :, :],
                                         wxp[:, j, :, :], 0.0,
                                         wya[:, b8, :, :],
                                         Alu.max, Alu.mult)

            # ---------- matmul 1: G = featx^T @ A  (accumulate over chunks)
            groups = [(0, 5), (5, 10), (10, 15), (15, 20), (20, 25),
                      (25, 30), (30, 33)]
            g_ps = []
            for gi, (c0, c1) in enumerate(groups):
                g_ps.append(psum.tile([17, (c1 - c0) * M_LOC], f32,
                                      tag=f"g{gi}", name=f"g{gi}"))
            for u in range(NCH):
                lhs = feat_s[:, u * 17:(u + 1) * 17].bitcast(f32r)
                for gi, (c0, c1) in enumerate(groups):
                    rhs = a_t[:, c0:c1, u, :].bitcast(f32r)
                    nc.tensor.matmul(g_ps[gi][:], lhs, rhs,
                                     start=(u == 0), stop=(u == NCH - 1))

            # ---------- psi -> 1/psi broadcast to 16 partitions ----------
            psi_ap = g_ps[6][16:17, 2 * M_LOC:3 * M_LOC]  # [1, 96]
            psir = pool.tile([1, M_LOC], f32, tag="psir", name="psir")
            V.tensor_scalar(psir[:], psi_ap, 1e-35, None, Alu.max)
            V.reciprocal(psir[:], psir[:])
            ones16 = pool.tile([1, 16], f32, tag="ones16", name="ones16")
            V.memset(ones16[:], 1.0)
            rm_ps = psum.tile([16, M_LOC], f32, tag="g0", name="rm_ps")
            nc.tensor.matmul(rm_ps[:], ones16[:], psir[:])

            # ---------- G PSUM -> SBUF (fat copies), then partition
            # relocation via SBUF->SBUF DMA for matmul 2 ----------
            gs = pool.tile([16, NCELL * M_LOC], f32, tag="gs", name="gs")
            for gi, (c0, c1) in enumerate(groups):
                w = (min(c1, NCELL) - c0) * M_LOC
                eng = V if gi % 2 == 0 else S
                if eng is S:
                    S.activation(gs[:, c0 * M_LOC:c0 * M_LOC + w],
                                 g_ps[gi][0:16, 0:w], Act.Copy)
                else:
                    V.tensor_copy(gs[:, c0 * M_LOC:c0 * M_LOC + w],
                                  g_ps[gi][0:16, 0:w])
            gt = pool.tile([128, 4, M_LOC], f32, tag="gt", name="gt")
            for cell in range(NCELL):
                q, cl = divmod(cell, 8)
                nc.sync.dma_start(
                    gt[cl * 16:(cl + 1) * 16, q, :],
                    gs[:, cell * M_LOC:(cell + 1) * M_LOC])

            # ---------- matmul 2: out2 = K2^T @ Gt ----------
            o2_ps = psum.tile([16, M_LOC], f32, tag="g1", name="o2_ps")
            for q in range(4):
                nc.tensor.matmul(o2_ps[:],
                                 k2_s[:, q * 16:(q + 1) * 16].bitcast(f32r),
                                 gt[:, q, :].bitcast(f32r),
                                 start=(q == 0), stop=(q == 3))

            # ---------- scale by 1/psi, store ----------
            out_s = pool.tile([16, M_LOC], f32, tag="outs", name="out_s")
            V.tensor_tensor(out_s[:], o2_ps[:], rm_ps[:], Alu.mult)
            nc.sync.dma_start(outd[:], out_s[:])

    return nc


def get_module(cfg=None):
    cfg = dict(CFG, **(cfg or {}))
    key = tuple(sorted((k, str(v)) for k, v in cfg.items()))
    if key not in _module_cache:
        _module_cache[key] = _build_module(cfg)
    return _module_cache[key]


def make_in_maps(field, center, field_feat, field_mask, kernel, cfg=None):
    """Host-side shard + layout prep. Returns list of 8 in_maps."""
    cfg = dict(CFG, **(cfg or {}))
    field = np.asarray(field, np.float32)
    center = np.asarray(center, np.float32)
    feat = np.asarray(field_feat, np.float32)
    mask = np.asarray(field_mask, np.float32)
    ker = np.asarray(kernel, np.float32)

    # K2big: [128 rows = (cell%8)*16 + (ci*2+x), 64 cols = (cell//8)*16 + (co*2+y)]
    kk = ker.transpose(3, 2, 1, 5, 0, 4).reshape(NCELL, 16, 16)  # [cell,(ci,x),(co,y)]
    k2b = kk.reshape(4, 8, 16, 16).transpose(1, 2, 0, 3).reshape(128, 64)
    k2b = np.ascontiguousarray(k2b, np.float32)

    in_maps = []
    for c in range(N_CORES):
        b, blk = divmod(c, 4)
        m0 = blk * M_LOC
        cx = center[b, m0:m0 + M_LOC, 0] / RADIUS   # [96]
        cy = center[b, m0:m0 + M_LOC, 1] / RADIUS
        fx = (field[b, :, 0] / RADIUS).reshape(NCH, 128).T  # [128, 3]
        fy = (field[b, :, 1] / RADIUS).reshape(NCH, 128).T

        if cfg["use_bcast"]:
            cb = np.concatenate([np.broadcast_to(cx, (128, M_LOC)),
                                 np.broadcast_to(cy, (128, M_LOC))], axis=1)
            f3 = np.concatenate([fx, fy], axis=1)
        else:
            cb = np.concatenate(
                [np.broadcast_to(cx, (128, FREE // M_LOC * M_LOC)).reshape(128, -1)[:, :0],  # unused
                 ], axis=1) if False else np.concatenate(
                [np.tile(np.broadcast_to(cx, (128, M_LOC)), (1, NCH)),
                 np.tile(np.broadcast_to(cy, (128, M_LOC)), (1, NCH))], axis=1)
            f3 = np.concatenate(
                [np.repeat(fx, M_LOC, axis=1),
                 np.repeat(fy, M_LOC, axis=1)], axis=1)

        fm = feat[b].reshape(N, 16) * mask[b]           # mask folded
        fcols = np.concatenate([fm, mask[b]], axis=1)   # [N, 17] ones-col*mask
        featx = fcols.reshape(NCH, 128, 17).transpose(1, 0, 2).reshape(128, 51)

        cst_row = np.array([-l for l in range(10)]
                           + [-(0.5 + j) for j in range(4)]
                           + [math.pi / 2, 1.0], np.float32)
        cst = np.broadcast_to(cst_row, (128, 16))
        in_maps.append({
            "cst": np.ascontiguousarray(cst, np.float32),
            "cb": np.ascontiguousarray(cb, np.float32),
            "f3": np.ascontiguousarray(f3, np.float32),
            "featx": np.ascontiguousarray(featx, np.float32),
            "k2b": k2b,
        })
    return in_maps


def unshard(results):
    out = np.zeros((B, M, CO, 2), np.float32)
    for c in range(N_CORES):
        b, blk = divmod(c, 4)
        m0 = blk * M_LOC
        out[b, m0:m0 + M_LOC] = results[c]["out"].T.reshape(M_LOC, CO, 2)
    return out


def kernel(field, center, field_feat, field_mask, kernel):
    from concourse.bass_utils import run_bass_kernel_spmd
    nc = get_module()
    in_maps = make_in_maps(field, center, field_feat, field_mask, kernel)
    res = run_bass_kernel_spmd(nc, in_maps, core_ids=list(range(N_CORES)))
    return unshard(res.results)


# revision 14
# speedup vs baseline: 1.4956x; 1.4956x over previous
"""Trainium2 Bass kernel for nn_EquiCtsConvBase (equivariant continuous conv).

Math reformulation (per batch b, center m, field point n):
  rel = (field[n] - center[m]) / RADIUS
  r, theta = polar(rel)
  Bilinear grid-sample of kernel[(co,ci,y,x), theta_pad, r] at
  (gx, gy) decomposes into separable hat functions:
    Wx[j]  = relu(1 - |4r - 0.5 - j|)            j = 0..3   (radius cells)
    Wy[l]  = relu(1 - |iy - l|), iy = 4*theta/pi + 4.5, l = 0..9
  Circular theta padding folds 10 rows -> 8 bins:
    Wy8[0] = max(Wy[1], Wy[9]); Wy8[7] = max(Wy[0], Wy[8]); Wy8[b]=Wy[b+1]
  att = relu(1 - |rel|^2)^3 * mask[n]   (mask folded into feat on host)
  A[(b8,j), n, m] = relu(Wx_pre[j]) * relu(Wy8_pre[b8]) * att
  G[m, cell, f]   = sum_n A[cell, n, m] * feat[n, f]        (PE matmul 1)
  out[m, (co,y)]  = sum_{cell,f} G * K2[cell, f, (co,y)]    (PE matmul 2)
  out /= max(psi, tiny), psi[m] = sum_n att[n, m]  (extra ones-column matmul)

theta is computed without a Sqrt (keeps a single ACT table, trig_and_small):
  phi = arctan(rely/relx);  theta = phi + pi*sign(rely)*[relx<0]
  r   = |relx*sin(phi+pi/2) + rely*sin(phi)|

Sharding: 8 cores; core c handles batch b = c//4, centers m0 = (c%4)*96 .. +96.
Each core's SPMD program is identical; only input data differs.
"""

import math
import numpy as np

RADIUS = 1.5
B, M, N = 2, 384, 384
CI = CO = 8
M_LOC = 96          # centers per core
NCH = 3             # n-chunks of 128 (N = 384)
NCELL = 32          # 8 theta bins x 4 radius cells
FREE = NCH * M_LOC  # 288: fused (chunk, m) free dim for elementwise ops
N_CORES = 8

# --- engine assignment tuning knobs ---
CFG = dict(
    wy_act=tuple(range(0, 8)),   # Wy hat indices computed on ACT (rest on DVE)
    n_a_gps=10,                  # how many of the 32 A-cell ops go to GPSIMD
    wya_gps=True,                # wya (8 stt ops) on GPSIMD
    use_bcast=True,              # stride-0 free-dim broadcast reads
    a_dtype="f32",               # dtype for A / feat matmul operands
)

_module_cache = {}


def _build_module(cfg):
    import concourse.bass as bass
    import concourse.mybir as mybir
    from concourse import tile

    dt = mybir.dt
    Alu = mybir.AluOpType
    Act = mybir.ActivationFunctionType

    nc = bass.Bass("TRN2", target_bir_lowering=False, debug=False,
                   num_devices=N_CORES)

    # ------------- DRAM I/O -------------
    if cfg["use_bcast"]:
        cbd = nc.dram_tensor("cb", [128, 2 * M_LOC], dt.float32,
                             kind="ExternalInput").ap()      # cx|cy per m
        f3d = nc.dram_tensor("f3", [128, 2 * NCH], dt.float32,
                             kind="ExternalInput").ap()      # fx|fy per chunk
    else:
        cbd = nc.dram_tensor("cb", [128, 2 * FREE], dt.float32,
                             kind="ExternalInput").ap()
        f3d = nc.dram_tensor("f3", [128, 2 * FREE], dt.float32,
                             kind="ExternalInput").ap()
    featd = nc.dram_tensor("featx", [128, NCH * 17], dt.float32,
                           kind="ExternalInput").ap()
    k2d = nc.dram_tensor("k2b", [16, NCELL * 16], dt.float32,
                         kind="ExternalInput").ap()
    cstd = nc.dram_tensor("cst", [128, 16], dt.float32,
                          kind="ExternalInput").ap()
    outd = nc.dram_tensor("out", [M_LOC, 16], dt.float32,
                          kind="ExternalOutput").ap()

    f32 = dt.float32
    f32r = dt.float32r

    with tile.TileContext(nc) as tc:
        with tc.tile_pool(name="p", bufs=1) as pool, \
             tc.tile_pool(name="ps", bufs=1, space="PSUM") as psum:

            # ---------- loads ----------
            cb_s = pool.tile(list(cbd.shape), f32, tag="cb", name="cb_s")
            f3_s = pool.tile(list(f3d.shape), f32, tag="f3", name="f3_s")
            feat_s = pool.tile([128, NCH * 17], f32, tag="feat", name="feat_s")
            k2_s = pool.tile([16, NCELL * 16], f32, tag="k2", name="k2_s")
            cst_s = pool.tile([128, 16], f32, tag="cst", name="cst_s")
            # spread loads across engine DMA queues so they run concurrently
            nc.sync.dma_start(cb_s[:], cbd[:])
            nc.gpsimd.dma_start(f3_s[:], f3d[:])
            nc.sync.dma_start(feat_s[:], featd[:])
            nc.scalar.dma_start(k2_s[:], k2d[:])
            nc.scalar.dma_start(cst_s[:], cstd[:])

            # const bias columns: 0..9 -> -l (Wy), 10..13 -> -(0.5+j) (Wx),
            # 14 -> pi/2, 15 -> 1.0
            def cB(i):
                return cst_s[:, i:i + 1]

            def wt(tag, shape=None):
                return pool.tile(shape or [128, NCH, M_LOC], f32, tag=tag,
                                 name=tag)

            # broadcast views [128, NCH, M_LOC]
            if cfg["use_bcast"]:
                cx_b = cb_s[:, None, 0:M_LOC].to_broadcast((128, NCH, M_LOC))
                cy_b = cb_s[:, None, M_LOC:2 * M_LOC].to_broadcast(
                    (128, NCH, M_LOC))
                fx_b = f3_s[:, 0:NCH, None].to_broadcast((128, NCH, M_LOC))
                fy_b = f3_s[:, NCH:2 * NCH, None].to_broadcast(
                    (128, NCH, M_LOC))
            else:
                cx_b = cb_s[:, 0:FREE].rearrange("p (u m) -> p u m", u=NCH)
                cy_b = cb_s[:, FREE:2 * FREE].rearrange(
                    "p (u m) -> p u m", u=NCH)
                fx_b = f3_s[:, 0:FREE].rearrange("p (u m) -> p u m", u=NCH)
                fy_b = f3_s[:, FREE:2 * FREE].rearrange(
                    "p (u m) -> p u m", u=NCH)

            V, S, G = nc.vector, nc.scalar, nc.gpsimd

            # ---------- elementwise stage ----------
            relx = wt("relx"); rely = wt("rely")
            V.tensor_tensor(relx[:], fx_b, cx_b, Alu.subtract)
            V.tensor_tensor(rely[:], fy_b, cy_b, Alu.subtract)

            sqx = wt("sqx"); sqy = wt("sqy"); rho = wt("rho")
            S.activation(sqx[:], relx[:], Act.Square)
            S.activation(sqy[:], rely[:], Act.Square)
            V.tensor_tensor(rho[:], sqx[:], sqy[:], Alu.add)

            # phi = atan(rely/relx) with range reduction to |arg| <= 1:
            #   t1 = y/x, t2 = x/y; swap where |t1| > 1;
            #   phi = swap ? sign(y)*sign(x)*pi/2 - atan(t2) : atan(t1)
            rx = wt("rx"); ry = wt("ry"); t1 = wt("t1"); t2 = wt("t2")
            swp = wt("swp"); phi = wt("phi"); psw = wt("psw")
            sgn = wt("sgn"); neg = wt("neg"); sgx = wt("sgx"); sgt = wt("sgt")
            V.reciprocal(rx[:], relx[:])
            V.reciprocal(ry[:], rely[:])
            V.tensor_tensor(t1[:], rely[:], rx[:], Alu.mult)
            V.tensor_tensor(t2[:], relx[:], ry[:], Alu.mult)
            V.tensor_scalar(swp[:], t1[:], 0.0, 1.0, Alu.abs_max, Alu.is_gt)
            V.tensor_copy(phi[:], t1[:])
            V.copy_predicated(phi[:], swp[:], t2[:])
            S.activation(phi[:], phi[:], Act.Arctan)
            S.activation(sgn[:], rely[:], Act.Sign)
            V.tensor_scalar(neg[:], relx[:], 0.0, None, Alu.is_lt)
            V.tensor_scalar(sgx[:], neg[:], -2.0, 1.0, Alu.mult, Alu.add)
            V.tensor_tensor(sgt[:], sgn[:], sgx[:], Alu.mult)
            V.scalar_tensor_tensor(psw[:], sgt[:], math.pi / 2, phi[:],
                                   Alu.mult, Alu.subtract)
            V.copy_predicated(phi[:], swp[:], psw[:])

            cs = wt("cs"); sn = wt("sn")
            S.activation(cs[:], phi[:], Act.Sin, bias=cB(14))
            S.activation(sn[:], phi[:], Act.Sin)

            xc = wt("xc"); ys = wt("ys"); rr = wt("rr")
            V.tensor_tensor(xc[:], relx[:], cs[:], Alu.mult)
            V.tensor_tensor(ys[:], rely[:], sn[:], Alu.mult)
            V.tensor_tensor(rr[:], xc[:], ys[:], Alu.add)
            V.tensor_scalar(rr[:], rr[:], 0.0, None, Alu.abs_max)  # r = |.|

            corr = wt("corr"); phis = wt("phis"); iy = wt("iy")
            V.tensor_tensor(corr[:], sgn[:], neg[:], Alu.mult)
            V.tensor_scalar(phis[:], phi[:], 4.0 / math.pi, 4.5,
                            Alu.mult, Alu.add)
            V.scalar_tensor_tensor(iy[:], corr[:], 4.0, phis[:],
                                   Alu.mult, Alu.add)

            u1 = wt("u1"); u2 = wt("u2")
            S.activation(u1[:], rho[:], Act.Relu, bias=cB(15), scale=-1.0)
            V.tensor_tensor(u2[:], u1[:], u1[:], Alu.mult)

            # A tile: [128, 33 cells, NCH, M_LOC]; block 32 = att
            a_t = pool.tile([128, NCELL + 1, NCH, M_LOC], f32, tag="a_t", name="a_t")
            att = a_t[:, NCELL, :, :]
            V.scalar_tensor_tensor(att, u2[:], 1.0, u1[:],
                                   Alu.mult, Alu.mult)  # u1^3

            # Wy hats (pre-relu): wyh[l] = 1 - |iy - l|
            wyh = pool.tile([128, 10, NCH, M_LOC], f32, tag="wyh", name="wyh")
            for l in range(10):
                dst = wyh[:, l, :, :]
                if l in cfg["wy_act"]:
                    S.activation(dst, iy[:], Act.Abs, bias=cB(l))
                    S.activation(dst, dst, Act.Identity, bias=cB(15), scale=-1.0)
                else:
                    V.tensor_scalar(dst, iy[:], float(l), 0.0,
                                    Alu.subtract, Alu.abs_max)
                    V.tensor_scalar(dst, dst, -1.0, 1.0, Alu.mult, Alu.add)

            # fold to 8 bins (pre-relu; supports disjoint => max == sum)
            w0s = wt("w0s"); w7s = wt("w7s")
            V.tensor_tensor(w0s[:], wyh[:, 1, :, :], wyh[:, 9, :, :], Alu.max)
            V.tensor_tensor(w7s[:], wyh[:, 0, :, :], wyh[:, 8, :, :], Alu.max)

            def wy8_pre(b8):
                if b8 == 0:
                    return w0s[:]
                if b8 == 7:
                    return w7s[:]
                return wyh[:, b8 + 1, :, :]

            # wya[b8] = relu(wy8_pre) * att
            wya = pool.tile([128, 8, NCH, M_LOC], f32, tag="wya", name="wya")
            eng_wya = G if cfg["wya_gps"] else V
            for b8 in range(8):
                eng_wya.scalar_tensor_tensor(wya[:, b8, :, :], wy8_pre(b8),
                                             0.0, att, Alu.max, Alu.mult)

            # Wx pre-relu hats on ACT: wxp[j] = 1 - |4r - (0.5+j)|
            wxp = pool.tile([128, 4, NCH, M_LOC], f32, tag="wxp", name="wxp")
            for j in range(4):
                dst = wxp[:, j, :, :]
                S.activation(dst, rr[:], Act.Abs, bias=cB(10 + j), scale=4.0)
                S.activation(dst, dst, Act.Identity, bias=cB(15), scale=-1.0)

            # A cells: A = relu(wx_pre) * wya
            n_gps = cfg["n_a_gps"]
            for cell in range(NCELL):
                b8, j = divmod(cell, 4)
                eng = G if cell < n_gps else V
                eng.scalar_tensor_tensor(a_t[:, cell, :, :],
                                         wxp[:, j, :, :], 0.0,
                                         wya[:, b8, :, :],
                                         Alu.max, Alu.mult)

            # ---------- matmul 1: G = featx^T @ A  (accumulate over chunks)
            groups = [(0, 5), (5, 10), (10, 15), (15, 20), (20, 25),
                      (25, 30), (30, 33)]
            g_ps = []
            for gi, (c0, c1) in enumerate(groups):
                g_ps.append(psum.tile([17, (c1 - c0) * M_LOC], f32,
                                      tag=f"g{gi}", name=f"g{gi}"))
            for u in range(NCH):
                lhs = feat_s[:, u * 17:(u + 1) * 17].bitcast(f32r)
                for gi, (c0, c1) in enumerate(groups):
                    rhs = a_t[:, c0:c1, u, :].bitcast(f32r)
                    nc.tensor.matmul(g_ps[gi][:], lhs, rhs,
                                     start=(u == 0), stop=(u == NCH - 1))

            # ---------- psi -> 1/psi, transposed to [96, 1] ----------
            psi_ap = g_ps[6][16:17, 2 * M_LOC:3 * M_LOC]  # [1, 96]
            psir = pool.tile([1, M_LOC], f32, tag="psir", name="psir")
            V.tensor_scalar(psir[:], psi_ap, 1e-35, None, Alu.max)
            V.reciprocal(psir[:], psir[:])
            psit = pool.tile([M_LOC, 1], f32, tag="psit", name="psit")
            nc.sync.dma_start(psit[:, 0:1], psir[0:1, :])

            # ---------- G PSUM -> SBUF (fat aligned copies) ----------
            gs = pool.tile([16, NCELL * M_LOC], f32, tag="gs", name="gs")
            for gi, (c0, c1) in enumerate(groups):
                w = (min(c1, NCELL) - c0) * M_LOC
                dst = gs[:, c0 * M_LOC:c0 * M_LOC + w]
                if gi % 2 == 0:
                    V.tensor_copy(dst, g_ps[gi][0:16, 0:w])
                else:
                    S.activation(dst, g_ps[gi][0:16, 0:w], Act.Copy)

            # ---------- matmul 2 (transposed): o2t[m, coy] ----------
            o2t = psum.tile([M_LOC, 16], f32, tag="o2t", name="o2t")
            for c in range(NCELL):
                nc.tensor.matmul(o2t[:],
                                 gs[:, c * M_LOC:(c + 1) * M_LOC],
                                 k2_s[:, c * 16:(c + 1) * 16],
                                 start=(c == 0), stop=(c == NCELL - 1))

            # ---------- scale by 1/psi, store ----------
            out_s = pool.tile([M_LOC, 16], f32, tag="outs", name="out_s")
            V.tensor_scalar(out_s[:], o2t[:], psit[:, 0:1], None, Alu.mult)
            nc.sync.dma_start(outd[:], out_s[:])

    return nc


def get_module(cfg=None):
    cfg = dict(CFG, **(cfg or {}))
    key = tuple(sorted((k, str(v)) for k, v in cfg.items()))
    if key not in _module_cache:
        _module_cache[key] = _build_module(cfg)
    return _module_cache[key]


def make_in_maps(field, center, field_feat, field_mask, kernel, cfg=None):
    """Host-side shard + layout prep. Returns list of 8 in_maps."""
    cfg = dict(CFG, **(cfg or {}))
    field = np.asarray(field, np.float32)
    center = np.asarray(center, np.float32)
    feat = np.asarray(field_feat, np.float32)
    mask = np.asarray(field_mask, np.float32)
    ker = np.asarray(kernel, np.float32)

    # K2big: [128 rows = (cell%8)*16 + (ci*2+x), 64 cols = (cell//8)*16 + (co*2+y)]
    kk = ker.transpose(3, 2, 1, 5, 0, 4).reshape(NCELL, 16, 16)  # [cell,(ci,x),(co,y)]
    k2b = np.ascontiguousarray(kk.transpose(1, 0, 2).reshape(16, NCELL * 16),
                               np.float32)

    in_maps = []
    for c in range(N_CORES):
        b, blk = divmod(c, 4)
        m0 = blk * M_LOC
        cx = center[b, m0:m0 + M_LOC, 0] / RADIUS   # [96]
        cy = center[b, m0:m0 + M_LOC, 1] / RADIUS
        fx = (field[b, :, 0] / RADIUS).reshape(NCH, 128).T  # [128, 3]
        fy = (field[b, :, 1] / RADIUS).reshape(NCH, 128).T

        if cfg["use_bcast"]:
            cb = np.concatenate([np.broadcast_to(cx, (128, M_LOC)),
                                 np.broadcast_to(cy, (128, M_LOC))], axis=1)
            f3 = np.concatenate([fx, fy], axis=1)
        else:
            cb = np.concatenate(
                [np.broadcast_to(cx, (128, FREE // M_LOC * M_LOC)).reshape(128, -1)[:, :0],  # unused
                 ], axis=1) if False else np.concatenate(
                [np.tile(np.broadcast_to(cx, (128, M_LOC)), (1, NCH)),
                 np.tile(np.broadcast_to(cy, (128, M_LOC)), (1, NCH))], axis=1)
            f3 = np.concatenate(
                [np.repeat(fx, M_LOC, axis=1),
                 np.repeat(fy, M_LOC, axis=1)], axis=1)

        fm = feat[b].reshape(N, 16) * mask[b]           # mask folded
        fcols = np.concatenate([fm, mask[b]], axis=1)   # [N, 17] ones-col*mask
        featx = fcols.reshape(NCH, 128, 17).transpose(1, 0, 2).reshape(128, 51)

        cst_row = np.array([-l for l in range(10)]
                           + [-(0.5 + j) for j in range(4)]
                           + [math.pi / 2, 1.0], np.float32)
        cst = np.broadcast_to(cst_row, (128, 16))
        in_maps.append({
            "cst": np.ascontiguousarray(cst, np.float32),
            "cb": np.ascontiguousarray(cb, np.float32),
            "f3": np.ascontiguousarray(f3, np.float32),
            "featx": np.ascontiguousarray(featx, np.float32),
            "k2b": k2b,
        })
    return in_maps


def unshard(results):
    out = np.zeros((B, M, CO, 2), np.float32)
    for c in range(N_CORES):
        b, blk = divmod(c, 4)
        m0 = blk * M_LOC
        out[b, m0:m0 + M_LOC] = results[c]["out"].reshape(M_LOC, CO, 2)
    return out


def kernel(field, center, field_feat, field_mask, kernel):
    from concourse.bass_utils import run_bass_kernel_spmd
    nc = get_module()
    in_maps = make_in_maps(field, center, field_feat, field_mask, kernel)
    res = run_bass_kernel_spmd(nc, in_maps, core_ids=list(range(N_CORES)))
    return unshard(res.results)
